# revision 1
# baseline (speedup 1.0000x reference)
"""Luong 'concat' attention TRN2 Bass kernel.

Problem: B=64, S=2048, D=512 (enc_dim == dec_dim), fp32.
  hidden = tanh(enc @ W_enc^T + ht @ W_dec^T + W_b)   [B, S, D]
  scores = hidden @ V_w^T (+ V_b)                     [B, 1, S]
  weights = softmax(scores, axis=-1)
  c_t = weights @ enc                                 [B, 1, D]

Sharding: data-parallel over batch, 8 batches per core on 8 cores.

Per-core dataflow (per batch, fully fused, enc read from HBM exactly once):
  1. DMA enc[b] natural-layout (contiguous, line-rate) into SBUF.
  2. PE transposes [128,128] -> encT (e on partitions), ACT/DVE copy-backs
     round PSUM -> SBUF float32r (required pre-rounding for f32r matmuls).
  3. mm1: hiddenT[d,s] psum tiles = sum_e W_encT^T @ encT (f32r, N=512,
     1 cyc/row).  Decoder bias folded in via ACT per-partition bias operand:
     tanh tile = activation(psum, Tanh, bias=(ht@W_dec^T + W_b)[d,b]).
  4. scores via V replicated across all 128 PE output rows: psum_s[128,512]
     every partition holds the same score row; exp on ACT with accum_out
     gives softmax denominator for free; V_b dropped (softmax shift-invariant).
  5. c_t[d] = sum_s u[s]*encT[d,s] as DVE fused tensor_tensor_reduce over
     the free dim, then scale by 1/denom (per-partition scalar).
"""

import numpy as np

B, S, D = 64, 2048, 512
N_CORES = 8
BPC = B // N_CORES      # batches per core
ET = D // 128           # 4 e-tiles (enc feature chunks)
DT = D // 128           # 4 d-tiles (output feature chunks)
SC = S // 512           # 4 s-windows of 512
SCJ = 4                 # 128-blocks per s-window

_CACHE = {}


def _build():
    import concourse.bacc as bacc
    import concourse.tile as tile
    from concourse import mybir
    from concourse.masks import make_identity

    f32 = mybir.dt.float32
    f32r = mybir.dt.float32r
    Tanh = mybir.ActivationFunctionType.Tanh
    Exp = mybir.ActivationFunctionType.Exp
    AX = mybir.AxisListType.X
    mult = mybir.AluOpType.mult
    add = mybir.AluOpType.add

    nc = bacc.Bacc(None, target_bir_lowering=False, debug=False)
    enc = nc.dram_tensor("enc_outs", [BPC, S, D], f32, kind="ExternalInput").ap()
    ht = nc.dram_tensor("ht", [1, BPC, D], f32, kind="ExternalInput").ap()
    W_w = nc.dram_tensor("W_w", [D, 2 * D], f32, kind="ExternalInput").ap()
    W_b = nc.dram_tensor("W_b", [D], f32, kind="ExternalInput").ap()
    V_w = nc.dram_tensor("V_w", [1, D], f32, kind="ExternalInput").ap()
    out = nc.dram_tensor("c_t", [BPC, 1, D], f32, kind="ExternalOutput").ap()

    with tile.TileContext(nc) as tc:
        with tc.tile_pool(name="const", bufs=1) as const, \
             tc.tile_pool(name="enc_nat", bufs=6) as enc_pool, \
             tc.tile_pool(name="encT", bufs=2 * ET) as encT_pool, \
             tc.tile_pool(name="hT", bufs=8) as hT_pool, \
             tc.tile_pool(name="u", bufs=2) as u_pool, \
             tc.tile_pool(name="scratch", bufs=1) as scratch_pool, \
             tc.tile_pool(name="small", bufs=8) as small_pool, \
             tc.tile_pool(name="pp_t", bufs=3, space="PSUM") as pp_t, \
             tc.tile_pool(name="pp_h", bufs=3, space="PSUM") as pp_h, \
             tc.tile_pool(name="pp_s", bufs=1, space="PSUM") as pp_s, \
             tc.tile_pool(name="pp_sm", bufs=1, space="PSUM") as pp_sm:

            ident = const.tile([128, 128], f32)
            make_identity(nc, ident)
            ident_r = const.tile([128, 128], f32r)
            nc.scalar.copy(out=ident_r, in_=ident)

            # ---- weights: load natural, transpose on PE, round to f32r ----
            wnat = const.tile([128, DT, 2 * D], f32)   # W_w rows d=dc*128+p
            nc.sync.dma_start(
                out=wnat, in_=W_w.rearrange("(dc p) e -> p dc e", p=128)
            )
            w_encT = [const.tile([128, D], f32r, tag=f"w_encT{i}", name=f"w_encT{i}") for i in range(ET)]
            w_decT = [const.tile([128, D], f32r, tag=f"w_decT{i}", name=f"w_decT{i}") for i in range(ET)]
            for ec in range(2 * ET):
                pt = pp_t.tile([128, D], f32, tag="pt")
                for dc in range(DT):
                    nc.tensor.transpose(
                        out=pt[:, dc * 128:(dc + 1) * 128],
                        in_=wnat[:, dc, ec * 128:(ec + 1) * 128],
                        identity=ident,
                    )
                dst = w_encT[ec] if ec < ET else w_decT[ec - ET]
                nc.scalar.copy(out=dst, in_=pt)

            # ---- V: [1,512] -> per-partition cols -> replicated f32r tiles ----
            vrow = const.tile([1, D], f32)
            nc.sync.dma_start(out=vrow, in_=V_w)
            v_pcol = const.tile([128, DT], f32)
            for dt_i in range(DT):
                pv = pp_sm.tile([128, 8], f32, tag="pp_small")
                nc.tensor.transpose(
                    out=pv[:, 0:1],
                    in_=vrow[0:1, dt_i * 128:(dt_i + 1) * 128],
                    identity=ident[0:1, 0:1],
                )
                nc.scalar.copy(out=v_pcol[:, dt_i:dt_i + 1], in_=pv[:, 0:1])
            import concourse.bass as bass
            v_rep = [const.tile([128, 128], f32r, tag=f"v_rep{i}", name=f"v_rep{i}") for i in range(DT)]
            for dt_i in range(DT):
                src = v_pcol[:, dt_i:dt_i + 1]
                src_b = bass.AP(tensor=src.tensor, offset=src.offset,
                                ap=[src.ap[0], [0, 128]])
                nc.scalar.copy(out=v_rep[dt_i], in_=src_b)

            # ---- W_b -> per-partition cols ----
            wbrow = const.tile([1, D], f32)
            nc.sync.dma_start(out=wbrow, in_=W_b.rearrange("(o d) -> o d", o=1))
            wb_pcol = const.tile([128, DT], f32)
            for dc in range(DT):
                pv = pp_sm.tile([128, 8], f32, tag="pp_small")
                nc.tensor.transpose(
                    out=pv[:, 0:1],
                    in_=wbrow[0:1, dc * 128:(dc + 1) * 128],
                    identity=ident[0:1, 0:1],
                )
                nc.scalar.copy(out=wb_pcol[:, dc:dc + 1], in_=pv[:, 0:1])

            # ---- ht -> htT (f32r) -> decoder bias  bias_db[dc][d, b] ----
            htn = const.tile([BPC, D], f32)
            nc.sync.dma_start(out=htn, in_=ht[0])
            htT = const.tile([128, ET, BPC], f32r)
            for ec in range(ET):
                pv = pp_sm.tile([128, 8], f32, tag="pp_small")
                nc.tensor.transpose(
                    out=pv[:, 0:BPC],
                    in_=htn[:, ec * 128:(ec + 1) * 128],
                    identity=ident[0:BPC, 0:BPC],
                )
                nc.scalar.copy(out=htT[:, ec, :], in_=pv[:, 0:BPC])
            bias_db = [const.tile([128, BPC], f32, tag=f"bias{i}", name=f"bias{i}") for i in range(DT)]
            for dc in range(DT):
                pb = pp_sm.tile([128, 8], f32, tag="pp_small")
                for ec in range(ET):
                    nc.tensor.matmul(
                        out=pb[:, 0:BPC],
                        lhsT=w_decT[ec][:, dc * 128:(dc + 1) * 128],
                        rhs=htT[:, ec, :],
                        start=(ec == 0), stop=(ec == ET - 1),
                    )
                nc.vector.tensor_scalar_add(
                    out=bias_db[dc], in0=pb[:, 0:BPC], scalar1=wb_pcol[:, dc:dc + 1]
                )

            # ---- per-batch accumulators for the final output ----
            ct_all = const.tile([128, BPC, ET], f32)

            # ================= main loop =================
            for b in range(BPC):
                encT = [encT_pool.tile([128, S], f32r, tag="encT", name=f"encT_b{b}_{_i}") for _i in range(ET)]
                u_all = u_pool.tile([128, S], f32)
                den4 = small_pool.tile([128, SC], f32, tag="den4")

                for sc in range(SC):
                    en = enc_pool.tile([128, SCJ, 512], f32r, tag="en")
                    nc.sync.dma_start(
                        out=en,
                        in_=enc[b, sc * 512:(sc + 1) * 512, :].rearrange(
                            "(scj p) e -> p scj e", p=128
                        ).bitcast(f32r),
                    )
                    # transpose to encT, rounding copy to f32r (split ACT/DVE)
                    for et in range(ET):
                        pt = pp_t.tile([128, 512], f32r, tag="pt")
                        for scj in range(SCJ):
                            nc.tensor.transpose(
                                out=pt[:, scj * 128:(scj + 1) * 128],
                                in_=en[:, scj, et * 128:(et + 1) * 128],
                                identity=ident_r,
                            )
                        dst = encT[et][:, sc * 512:(sc + 1) * 512]
                        if et % 2 == 0:
                            nc.scalar.copy(out=dst, in_=pt)
                        else:
                            nc.vector.tensor_copy(out=dst, in_=pt)

                    # mm1 + fused bias/tanh
                    hT = []
                    for dt_i in range(DT):
                        ph = pp_h.tile([128, 512], f32, tag="ph")
                        for et in range(ET):
                            nc.tensor.matmul(
                                out=ph,
                                lhsT=w_encT[et][:, dt_i * 128:(dt_i + 1) * 128],
                                rhs=encT[et][:, sc * 512:(sc + 1) * 512],
                                start=(et == 0), stop=(et == ET - 1),
                            )
                        h = hT_pool.tile([128, 512], f32r, tag="hT")
                        nc.scalar.activation(
                            out=h, in_=ph, func=Tanh, bias=bias_db[dt_i][:, b:b + 1]
                        )
                        hT.append(h)

                    # scores, replicated across partitions; exp + denom
                    ps = pp_s.tile([128, 512], f32, tag="ps")
                    for dt_i in range(DT):
                        nc.tensor.matmul(
                            out=ps, lhsT=v_rep[dt_i], rhs=hT[dt_i],
                            start=(dt_i == 0), stop=(dt_i == DT - 1),
                        )
                    nc.scalar.activation(
                        out=u_all[:, sc * 512:(sc + 1) * 512], in_=ps, func=Exp,
                        accum_out=den4[:, sc:sc + 1],
                    )

                # softmax denominator + reciprocal
                den = small_pool.tile([128, 1], f32, tag="den")
                nc.vector.tensor_reduce(out=den, in_=den4, axis=AX, op=add)
                rden = small_pool.tile([128, 1], f32, tag="rden")
                nc.vector.reciprocal(out=rden, in_=den)

                # c_t: fused multiply + free-dim reduce, then normalize
                ctb = small_pool.tile([128, ET], f32, tag="ctb")
                for et in range(ET):
                    scr = scratch_pool.tile([128, S], f32, tag="scr")
                    nc.vector.scalar_tensor_tensor(
                        out=scr,
                        in0=encT[et].bitcast(f32),
                        scalar=1.0,
                        in1=u_all,
                        op0=mult,
                        op1=mult,
                        accum_out=ctb[:, et:et + 1],
                    )
                nc.vector.tensor_scalar_mul(
                    out=ct_all[:, b, :], in0=ctb, scalar1=rden
                )

            # ---- output: transpose [128 p, (b et)] -> [(b et), p], one DMA ----
            pf = pp_t.tile([BPC * ET, 128], f32, tag="pt")
            nc.tensor.transpose(
                out=pf,
                in_=ct_all.rearrange("p b e -> p (b e)"),
                identity=ident,
            )
            ct_out = const.tile([BPC * ET, 128], f32)
            nc.scalar.copy(out=ct_out, in_=pf)
            nc.sync.dma_start(
                out=out.rearrange("b o (et p) -> (b o et) p", p=128),
                in_=ct_out,
            )

    nc.compile()
    return nc


def _get_nc():
    if "nc" not in _CACHE:
        _CACHE["nc"] = _build()
    return _CACHE["nc"]


def _run(inputs, trace=False, **kw):
    from concourse.bass_utils import run_bass_kernel_spmd

    nc = _get_nc()
    enc = np.asarray(inputs["enc_outs"], dtype=np.float32)
    ht = np.asarray(inputs["ht"], dtype=np.float32)
    W_w = np.asarray(inputs["W_w"], dtype=np.float32)
    W_b = np.asarray(inputs["W_b"], dtype=np.float32)
    V_w = np.asarray(inputs["V_w"], dtype=np.float32)
    in_maps = []
    for c in range(N_CORES):
        sl = slice(c * BPC, (c + 1) * BPC)
        in_maps.append({
            "enc_outs": enc[sl],
            "ht": ht[:, sl],
            "W_w": W_w,
            "W_b": W_b,
            "V_w": V_w,
        })
    res = run_bass_kernel_spmd(
        nc, in_maps, core_ids=list(range(N_CORES)), trace=trace, **kw
    )
    full = np.concatenate([res.results[c]["c_t"] for c in range(N_CORES)], axis=0)
    return full, res


def kernel(**inputs) -> np.ndarray:
    out, _ = _run(inputs, trace=False)
    return out



# revision 12
# speedup vs baseline: 1.7154x; 1.7154x over previous
"""Luong 'concat' attention TRN2 Bass kernel (fp8 DoubleRow edition).

Problem: B=64, S=2048, D=512 (enc_dim == dec_dim), fp32.
  hidden = tanh(enc @ W_enc^T + ht @ W_dec^T + W_b)   [B, S, D]
  scores = hidden @ V_w^T (+ V_b)                     [B, 1, S]
  weights = softmax(scores, axis=-1)
  c_t = weights @ enc                                 [B, 1, D]

Sharding: data-parallel over batch, 8 batches per core on 8 cores.

Per-core dataflow (per batch; enc read from HBM exactly once):
  1. DMA enc[b] natural-layout into SBUF (f32r bitcast).
  2. PE transposes [128,128] -> encT; ACT/DVE copy-backs convert the PSUM
     f32 to fp8e4 (enc ~ N(0,1) fits e4m3 natively).
  3. mm1 as fp8e4 DoubleRow matmuls (K_eff=256 per matmul, 2 per (dt,sc)):
     hiddenT[d,s] psum; W_enc pre-scaled x32 into fp8 (keeps small weights
     out of the subnormal range), un-scaled by ACT: tanh(psum/32 + bias).
     Decoder bias (ht@W_dec^T + W_b) folded in as ACT per-partition bias.
  4. scores computed as COLUMNS: for each 128-s block, out[128s,1] =
     hT_block^T @ v_col accumulated over the 4 d-chunks. Output free size
     is 1, so these matmuls are nearly free.  exp on ACT -> u_col[128,16].
  5. softmax denominator: DVE free-dim reduce of u_col, then a ones-matmul
     replicates the cross-partition sum to all 128 partitions; reciprocal
     on DVE.
  6. c_t as COLUMNS too: out[128e,1] = en_nat^T @ u_col_t accumulated over
     the 16 s-tiles (again free-size-1 matmuls vs natural-layout enc f32r,
     which keeps full precision on the value path).  Normalization by
     1/den happens in the tiny [128,4] psum->SBUF copy on DVE.
  7. One final PE transpose assembles all 8 batches' c_t columns into the
     natural [32,128] output layout; single DMA out.
"""

import numpy as np

B, S, D = 64, 2048, 512
N_CORES = 8
BPC = B // N_CORES      # batches per core
ET = D // 128           # 4 e-tiles (enc feature chunks)
DT = D // 128           # 4 d-tiles (output feature chunks)
SC = S // 512           # 4 s-windows of 512
SCJ = 4                 # 128-blocks per s-window
W_SCALE = 32.0          # fp8 pre-scale for W_enc

_CACHE = {}


def _build():
    import concourse.bacc as bacc
    import concourse.bass as bass
    import concourse.tile as tile
    from concourse import mybir
    from concourse.masks import make_identity

    f32 = mybir.dt.float32
    f32r = mybir.dt.float32r
    bf16 = mybir.dt.bfloat16
    f8 = mybir.dt.float8e4
    Tanh = mybir.ActivationFunctionType.Tanh
    Exp = mybir.ActivationFunctionType.Exp
    AX = mybir.AxisListType.X
    add = mybir.AluOpType.add
    DR = mybir.MatmulPerfMode.DoubleRow

    nc = bacc.Bacc(None, target_bir_lowering=False, debug=False)
    enc = nc.dram_tensor("enc_outs", [BPC, S, D], f32, kind="ExternalInput").ap()
    ht = nc.dram_tensor("ht", [1, BPC, D], f32, kind="ExternalInput").ap()
    W_w = nc.dram_tensor("W_w", [D, 2 * D], f32, kind="ExternalInput").ap()
    W_b = nc.dram_tensor("W_b", [D], f32, kind="ExternalInput").ap()
    V_w = nc.dram_tensor("V_w", [1, D], f32, kind="ExternalInput").ap()
    out = nc.dram_tensor("c_t", [BPC, 1, D], f32, kind="ExternalOutput").ap()

    with tile.TileContext(nc) as tc:
        with tc.tile_pool(name="const", bufs=1) as const, \
             tc.tile_pool(name="enc_nat", bufs=8) as enc_pool, \
             tc.tile_pool(name="encT", bufs=2) as encT_pool, \
             tc.tile_pool(name="hT", bufs=12) as hT_pool, \
             tc.tile_pool(name="small", bufs=8) as small_pool, \
             tc.tile_pool(name="pp_t", bufs=2, space="PSUM") as pp_t, \
             tc.tile_pool(name="pp_h", bufs=2, space="PSUM") as pp_h, \
             tc.tile_pool(name="pp_sm", bufs=2, space="PSUM") as pp_sm:

            ident = const.tile([128, 128], f32)
            make_identity(nc, ident)
            ident_r = const.tile([128, 128], f32r)
            nc.scalar.copy(out=ident_r, in_=ident)
            ones_f = const.tile([128, 128], f32)
            nc.gpsimd.memset(ones_f, 1.0)
            ones_r = const.tile([128, 128], f32r)
            nc.vector.tensor_copy(out=ones_r, in_=ones_f)

            # ---- weights: load natural [d-part, e'], transpose on PE ----
            wnat = const.tile([128, DT, 2 * D], f32)   # W_w rows d=dc*128+p
            nc.sync.dma_start(
                out=wnat, in_=W_w.rearrange("(dc p) e -> p dc e", p=128)
            )
            # enc half -> fp8 (x32); dec half -> f32r (for the bias matmul)
            w8 = const.tile([128, ET, D], f8, name="w8")
            wdT = const.tile([128, ET, D], f32r, name="wdT")
            for ec in range(2 * ET):
                ptw = pp_t.tile([128, D], f32, tag="pt")
                for dc in range(DT):
                    nc.tensor.transpose(
                        out=ptw[:, dc * 128:(dc + 1) * 128],
                        in_=wnat[:, dc, ec * 128:(ec + 1) * 128],
                        identity=ident,
                    )
                if ec < ET:
                    nc.scalar.mul(out=w8[:, ec, :], in_=ptw, mul=W_SCALE)
                else:
                    nc.scalar.copy(out=wdT[:, ec - ET, :], in_=ptw)

            # ---- V: [1,512] -> bf16 per-partition columns v_col[d,dt] ----
            vrow = const.tile([1, D], f32)
            nc.sync.dma_start(out=vrow, in_=V_w)
            v_col = const.tile([128, DT], bf16)
            for dt_i in range(DT):
                pv = pp_sm.tile([128, 16], f32, tag="sm")
                nc.tensor.transpose(
                    out=pv[:, 0:1],
                    in_=vrow[0:1, dt_i * 128:(dt_i + 1) * 128],
                    identity=ident[0:1, 0:1],
                )
                nc.scalar.copy(out=v_col[:, dt_i:dt_i + 1], in_=pv[:, 0:1])

            # ---- W_b -> per-partition cols ----
            wbrow = const.tile([1, D], f32)
            nc.sync.dma_start(out=wbrow, in_=W_b.rearrange("(o d) -> o d", o=1))
            wb_pcol = const.tile([128, DT], f32)
            for dc in range(DT):
                pv = pp_sm.tile([128, 16], f32, tag="sm")
                nc.tensor.transpose(
                    out=pv[:, 0:1],
                    in_=wbrow[0:1, dc * 128:(dc + 1) * 128],
                    identity=ident[0:1, 0:1],
                )
                nc.scalar.copy(out=wb_pcol[:, dc:dc + 1], in_=pv[:, 0:1])

            # ---- ht -> htT (f32r) -> decoder bias  bias_db[d, dt, b] ----
            htn = const.tile([BPC, D], f32)
            nc.sync.dma_start(out=htn, in_=ht[0])
            htT = const.tile([128, ET, BPC], f32r)
            for ec in range(ET):
                pv = pp_sm.tile([128, 16], f32, tag="sm")
                nc.tensor.transpose(
                    out=pv[:, 0:BPC],
                    in_=htn[:, ec * 128:(ec + 1) * 128],
                    identity=ident[0:BPC, 0:BPC],
                )
                nc.scalar.copy(out=htT[:, ec, :], in_=pv[:, 0:BPC])
            bias_db = const.tile([128, DT, BPC], f32, name="bias_db")
            for dc in range(DT):
                pb = pp_sm.tile([128, 16], f32, tag="sm")
                for ec in range(ET):
                    nc.tensor.matmul(
                        out=pb[:, 0:BPC],
                        lhsT=wdT[:, ec, dc * 128:(dc + 1) * 128],
                        rhs=htT[:, ec, :],
                        start=(ec == 0), stop=(ec == ET - 1),
                    )
                nc.vector.tensor_scalar_add(
                    out=bias_db[:, dc, :], in0=pb[:, 0:BPC],
                    scalar1=wb_pcol[:, dc:dc + 1]
                )

            # ---- per-batch c_t columns collected here ----
            ct_all = const.tile([128, BPC, ET], f32r)

            # ================= main loop =================
            for b in range(BPC):
                en = []
                encT8 = encT_pool.tile([128, ET, S], f8, tag="encT",
                                       name=f"encT_b{b}")
                for sc in range(SC):
                    e_t = enc_pool.tile([128, SCJ, 512], f32r, tag="en")
                    nc.sync.dma_start(
                        out=e_t,
                        in_=enc[b, sc * 512:(sc + 1) * 512, :].rearrange(
                            "(scj p) e -> p scj e", p=128
                        ).bitcast(f32r),
                    )
                    en.append(e_t)
                    # transpose to encT, converting to fp8 on copy-back
                    for et in range(ET):
                        pt = pp_t.tile([128, 512], f32r, tag="pt")
                        for scj in range(SCJ):
                            nc.tensor.transpose(
                                out=pt[:, scj * 128:(scj + 1) * 128],
                                in_=e_t[:, scj, et * 128:(et + 1) * 128],
                                identity=ident_r,
                            )
                        dst = encT8[:, et, sc * 512:(sc + 1) * 512]
                        if et == ET - 1 and sc % 2 == 0:
                            nc.scalar.copy(out=dst, in_=pt)
                        else:
                            nc.vector.tensor_copy(out=dst, in_=pt)

                # mm1 (fp8 DoubleRow) + fused bias/tanh; then score columns
                ps = pp_sm.tile([128, 16], f32, tag="sm")
                for scp in range(2):
                    hT = []
                    for dt_i in range(DT):
                        ph = pp_h.tile([128, 1024], f32, tag="ph")
                        for half in range(2):
                            sc = scp * 2 + half
                            for j in range(2):
                                nc.tensor.matmul(
                                    out=ph[:, half * 512:(half + 1) * 512],
                                    lhsT=w8[:, 2 * j:2 * j + 2,
                                            dt_i * 128:(dt_i + 1) * 128],
                                    rhs=encT8[:, 2 * j:2 * j + 2,
                                              sc * 512:(sc + 1) * 512],
                                    start=(j == 0), stop=(j == 1),
                                    perf_mode=DR,
                                )
                        h = hT_pool.tile([128, 1024], bf16, tag="hT")
                        nc.scalar.activation(
                            out=h, in_=ph, func=Tanh,
                            bias=bias_db[:, dt_i, b:b + 1],
                            scale=1.0 / W_SCALE,
                        )
                        hT.append(h)
                    for blk in range(8):
                        t = scp * 8 + blk
                        for dt_i in range(DT):
                            nc.tensor.matmul(
                                out=ps[:, t:t + 1],
                                lhsT=hT[dt_i][:, blk * 128:(blk + 1) * 128],
                                rhs=v_col[:, dt_i:dt_i + 1],
                                start=(dt_i == 0), stop=(dt_i == DT - 1),
                            )

                # softmax pieces
                u_col = small_pool.tile([128, 16], f32r, tag="ucol")
                nc.scalar.activation(out=u_col, in_=ps, func=Exp)
                den_p = small_pool.tile([128, 1], f32r, tag="denp")
                with nc.allow_low_precision(reason="f32r is full-width fp32"):
                    nc.vector.tensor_reduce(out=den_p, in_=u_col, axis=AX, op=add)
                # fp32r matmuls need an even moving-dim count: broadcast the
                # column to width 2 via a stride-0 AP (2nd output col unused).
                den_b = bass.AP(tensor=den_p.tensor, offset=den_p.offset,
                                ap=[den_p.ap[0], [0, 2]])
                pden = pp_sm.tile([128, 16], f32, tag="sm")
                nc.tensor.matmul(out=pden[:, 0:2], lhsT=ones_r, rhs=den_b,
                                 start=True, stop=True)
                rden = small_pool.tile([128, 1], f32, tag="rden")
                nc.vector.reciprocal(out=rden, in_=pden[:, 0:1])

                # c_t columns: accumulate over the 16 s-tiles
                pc = pp_sm.tile([128, 16], f32, tag="sm")
                for ec in range(ET):
                    for t in range(16):
                        sc, scj = t // 4, t % 4
                        u1 = u_col[:, t:t + 1]
                        u2 = bass.AP(tensor=u1.tensor, offset=u1.offset,
                                     ap=[u1.ap[0], [0, 2]])
                        nc.tensor.matmul(
                            out=pc[:, 2 * ec:2 * ec + 2],
                            lhsT=en[sc][:, scj, ec * 128:(ec + 1) * 128],
                            rhs=u2,
                            start=(t == 0), stop=(t == 15),
                        )
                pc_str = bass.AP(tensor=pc.tensor, offset=pc.offset,
                                 ap=[pc.ap[0], [2, ET]])
                nc.vector.tensor_scalar_mul(
                    out=ct_all[:, b, :], in0=pc_str, scalar1=rden
                )

            # ---- output: transpose [128 p, (b ec)] -> [(b ec), p], one DMA ----
            pf = pp_t.tile([BPC * ET, 128], f32r, tag="pt")
            nc.tensor.transpose(
                out=pf,
                in_=ct_all.rearrange("p b e -> p (b e)"),
                identity=ident_r,
            )
            ct_out = const.tile([BPC * ET, 128], f32)
            nc.vector.tensor_copy(out=ct_out, in_=pf)
            nc.sync.dma_start(
                out=out.rearrange("b o (et p) -> (b o et) p", p=128),
                in_=ct_out,
            )

    nc.compile()
    return nc


def _get_nc():
    if "nc" not in _CACHE:
        _CACHE["nc"] = _build()
    return _CACHE["nc"]


def _run(inputs, trace=False, **kw):
    from concourse.bass_utils import run_bass_kernel_spmd

    nc = _get_nc()
    enc = np.asarray(inputs["enc_outs"], dtype=np.float32)
    ht = np.asarray(inputs["ht"], dtype=np.float32)
    W_w = np.asarray(inputs["W_w"], dtype=np.float32)
    W_b = np.asarray(inputs["W_b"], dtype=np.float32)
    V_w = np.asarray(inputs["V_w"], dtype=np.float32)
    in_maps = []
    for c in range(N_CORES):
        sl = slice(c * BPC, (c + 1) * BPC)
        in_maps.append({
            "enc_outs": enc[sl],
            "ht": ht[:, sl],
            "W_w": W_w,
            "W_b": W_b,
            "V_w": V_w,
        })
    res = run_bass_kernel_spmd(
        nc, in_maps, core_ids=list(range(N_CORES)), trace=trace, **kw
    )
    full = np.concatenate([res.results[c]["c_t"] for c in range(N_CORES)], axis=0)
    return full, res


def kernel(**inputs) -> np.ndarray:
    out, _ = _run(inputs, trace=False)
    return out


# revision 15
# speedup vs baseline: 1.7164x; 1.0006x over previous
"""Luong 'concat' attention TRN2 Bass kernel (fp8 DoubleRow edition).

Problem: B=64, S=2048, D=512 (enc_dim == dec_dim), fp32.
  hidden = tanh(enc @ W_enc^T + ht @ W_dec^T + W_b)   [B, S, D]
  scores = hidden @ V_w^T (+ V_b)                     [B, 1, S]
  weights = softmax(scores, axis=-1)
  c_t = weights @ enc                                 [B, 1, D]

Sharding: data-parallel over batch, 8 batches per core on 8 cores.

Per-core dataflow (per batch; enc read from HBM exactly once):
  1. DMA enc[b] natural-layout into SBUF (f32r bitcast).
  2. PE transposes [128,128] -> encT; ACT/DVE copy-backs convert the PSUM
     f32 to fp8e4 (enc ~ N(0,1) fits e4m3 natively).
  3. mm1 as fp8e4 DoubleRow matmuls (K_eff=256 per matmul, 2 per (dt,sc)):
     hiddenT[d,s] psum; W_enc pre-scaled x32 into fp8 (keeps small weights
     out of the subnormal range), un-scaled by ACT: tanh(psum/32 + bias).
     Decoder bias (ht@W_dec^T + W_b) folded in as ACT per-partition bias.
  4. scores computed as COLUMNS: for each 128-s block, out[128s,1] =
     hT_block^T @ v_col accumulated over the 4 d-chunks. Output free size
     is 1, so these matmuls are nearly free.  exp on ACT -> u_col[128,16].
  5. softmax denominator: DVE free-dim reduce of u_col, then a ones-matmul
     replicates the cross-partition sum to all 128 partitions; reciprocal
     on DVE.
  6. c_t as COLUMNS too: out[128e,1] = en_nat^T @ u_col_t accumulated over
     the 16 s-tiles (again free-size-1 matmuls vs natural-layout enc f32r,
     which keeps full precision on the value path).  Normalization by
     1/den happens in the tiny [128,4] psum->SBUF copy on DVE.
  7. One final PE transpose assembles all 8 batches' c_t columns into the
     natural [32,128] output layout; single DMA out.
"""

import numpy as np

B, S, D = 64, 2048, 512
N_CORES = 8
BPC = B // N_CORES      # batches per core
ET = D // 128           # 4 e-tiles (enc feature chunks)
DT = D // 128           # 4 d-tiles (output feature chunks)
SC = S // 512           # 4 s-windows of 512
SCJ = 4                 # 128-blocks per s-window
W_SCALE = 32.0          # fp8 pre-scale for W_enc

_CACHE = {}


def _build():
    import concourse.bacc as bacc
    import concourse.bass as bass
    import concourse.tile as tile
    from concourse import mybir
    from concourse.masks import make_identity

    f32 = mybir.dt.float32
    f32r = mybir.dt.float32r
    bf16 = mybir.dt.bfloat16
    f8 = mybir.dt.float8e4
    Tanh = mybir.ActivationFunctionType.Tanh
    Exp = mybir.ActivationFunctionType.Exp
    AX = mybir.AxisListType.X
    add = mybir.AluOpType.add
    DR = mybir.MatmulPerfMode.DoubleRow

    nc = bacc.Bacc(None, target_bir_lowering=False, debug=False)
    enc = nc.dram_tensor("enc_outs", [BPC, S, D], f32, kind="ExternalInput").ap()
    ht = nc.dram_tensor("ht", [1, BPC, D], f32, kind="ExternalInput").ap()
    W_w = nc.dram_tensor("W_w", [D, 2 * D], f32, kind="ExternalInput").ap()
    W_b = nc.dram_tensor("W_b", [D], f32, kind="ExternalInput").ap()
    V_w = nc.dram_tensor("V_w", [1, D], f32, kind="ExternalInput").ap()
    out = nc.dram_tensor("c_t", [BPC, 1, D], f32, kind="ExternalOutput").ap()

    with tile.TileContext(nc) as tc:
        with tc.tile_pool(name="const", bufs=1) as const, \
             tc.tile_pool(name="enc_nat", bufs=10) as enc_pool, \
             tc.tile_pool(name="encT", bufs=3) as encT_pool, \
             tc.tile_pool(name="hT", bufs=16) as hT_pool, \
             tc.tile_pool(name="small", bufs=8) as small_pool, \
             tc.tile_pool(name="pp_t", bufs=2, space="PSUM") as pp_t, \
             tc.tile_pool(name="pp_h", bufs=2, space="PSUM") as pp_h, \
             tc.tile_pool(name="pp_sm", bufs=2, space="PSUM") as pp_sm:

            ident = const.tile([128, 128], f32)
            make_identity(nc, ident)
            ident_r = const.tile([128, 128], f32r)
            nc.scalar.copy(out=ident_r, in_=ident)
            ones_f = const.tile([128, 128], f32)
            nc.gpsimd.memset(ones_f, 1.0)
            ones_r = const.tile([128, 128], f32r)
            nc.vector.tensor_copy(out=ones_r, in_=ones_f)

            # ---- weights: load natural [d-part, e'], transpose on PE ----
            wnat = const.tile([128, DT, 2 * D], f32)   # W_w rows d=dc*128+p
            nc.sync.dma_start(
                out=wnat, in_=W_w.rearrange("(dc p) e -> p dc e", p=128)
            )
            # enc half -> fp8 (x32); dec half -> f32r (for the bias matmul)
            w8 = const.tile([128, ET, D], f8, name="w8")
            wdT = const.tile([128, ET, D], f32r, name="wdT")
            for ec in range(2 * ET):
                ptw = pp_t.tile([128, D], f32, tag="pt")
                for dc in range(DT):
                    nc.tensor.transpose(
                        out=ptw[:, dc * 128:(dc + 1) * 128],
                        in_=wnat[:, dc, ec * 128:(ec + 1) * 128],
                        identity=ident,
                    )
                if ec < ET:
                    nc.scalar.mul(out=w8[:, ec, :], in_=ptw, mul=W_SCALE)
                else:
                    nc.scalar.copy(out=wdT[:, ec - ET, :], in_=ptw)

            # ---- V: [1,512] -> bf16 per-partition columns v_col[d,dt] ----
            vrow = const.tile([1, D], f32)
            nc.sync.dma_start(out=vrow, in_=V_w)
            v_col = const.tile([128, DT], bf16)
            for dt_i in range(DT):
                pv = pp_sm.tile([128, 16], f32, tag="sm")
                nc.tensor.transpose(
                    out=pv[:, 0:1],
                    in_=vrow[0:1, dt_i * 128:(dt_i + 1) * 128],
                    identity=ident[0:1, 0:1],
                )
                nc.scalar.copy(out=v_col[:, dt_i:dt_i + 1], in_=pv[:, 0:1])

            # ---- W_b -> per-partition cols ----
            wbrow = const.tile([1, D], f32)
            nc.sync.dma_start(out=wbrow, in_=W_b.rearrange("(o d) -> o d", o=1))
            wb_pcol = const.tile([128, DT], f32)
            for dc in range(DT):
                pv = pp_sm.tile([128, 16], f32, tag="sm")
                nc.tensor.transpose(
                    out=pv[:, 0:1],
                    in_=wbrow[0:1, dc * 128:(dc + 1) * 128],
                    identity=ident[0:1, 0:1],
                )
                nc.scalar.copy(out=wb_pcol[:, dc:dc + 1], in_=pv[:, 0:1])

            # ---- ht -> htT (f32r) -> decoder bias  bias_db[d, dt, b] ----
            htn = const.tile([BPC, D], f32)
            nc.sync.dma_start(out=htn, in_=ht[0])
            htT = const.tile([128, ET, BPC], f32r)
            for ec in range(ET):
                pv = pp_sm.tile([128, 16], f32, tag="sm")
                nc.tensor.transpose(
                    out=pv[:, 0:BPC],
                    in_=htn[:, ec * 128:(ec + 1) * 128],
                    identity=ident[0:BPC, 0:BPC],
                )
                nc.scalar.copy(out=htT[:, ec, :], in_=pv[:, 0:BPC])
            bias_db = const.tile([128, DT, BPC], f32, name="bias_db")
            for dc in range(DT):
                pb = pp_sm.tile([128, 16], f32, tag="sm")
                for ec in range(ET):
                    nc.tensor.matmul(
                        out=pb[:, 0:BPC],
                        lhsT=wdT[:, ec, dc * 128:(dc + 1) * 128],
                        rhs=htT[:, ec, :],
                        start=(ec == 0), stop=(ec == ET - 1),
                    )
                nc.vector.tensor_scalar_add(
                    out=bias_db[:, dc, :], in0=pb[:, 0:BPC],
                    scalar1=wb_pcol[:, dc:dc + 1]
                )

            # ---- per-batch c_t columns collected here ----
            ct_all = const.tile([128, BPC, ET], f32r)

            # ================= main loop =================
            for b in range(BPC):
                en = []
                encT8 = encT_pool.tile([128, ET, S], f8, tag="encT",
                                       name=f"encT_b{b}")
                for sc in range(SC):
                    e_t = enc_pool.tile([128, SCJ, 512], f32r, tag="en")
                    nc.sync.dma_start(
                        out=e_t,
                        in_=enc[b, sc * 512:(sc + 1) * 512, :].rearrange(
                            "(scj p) e -> p scj e", p=128
                        ).bitcast(f32r),
                    )
                    en.append(e_t)
                    # transpose to encT, converting to fp8 on copy-back
                    for et in range(ET):
                        pt = pp_t.tile([128, 512], f32r, tag="pt")
                        for scj in range(SCJ):
                            nc.tensor.transpose(
                                out=pt[:, scj * 128:(scj + 1) * 128],
                                in_=e_t[:, scj, et * 128:(et + 1) * 128],
                                identity=ident_r,
                            )
                        dst = encT8[:, et, sc * 512:(sc + 1) * 512]
                        if et == ET - 1 and sc % 2 == 0:
                            nc.scalar.copy(out=dst, in_=pt)
                        else:
                            nc.vector.tensor_copy(out=dst, in_=pt)

                # mm1 (fp8 DoubleRow) + fused bias/tanh; then score columns
                ps = pp_sm.tile([128, 16], f32, tag="sm")
                for scp in range(2):
                    hT = []
                    for dt_i in range(DT):
                        ph = pp_h.tile([128, 1024], f32, tag="ph")
                        for half in range(2):
                            sc = scp * 2 + half
                            for j in range(2):
                                nc.tensor.matmul(
                                    out=ph[:, half * 512:(half + 1) * 512],
                                    lhsT=w8[:, 2 * j:2 * j + 2,
                                            dt_i * 128:(dt_i + 1) * 128],
                                    rhs=encT8[:, 2 * j:2 * j + 2,
                                              sc * 512:(sc + 1) * 512],
                                    start=(j == 0), stop=(j == 1),
                                    perf_mode=DR,
                                )
                        h = hT_pool.tile([128, 1024], bf16, tag="hT")
                        nc.scalar.activation(
                            out=h, in_=ph, func=Tanh,
                            bias=bias_db[:, dt_i, b:b + 1],
                            scale=1.0 / W_SCALE,
                        )
                        hT.append(h)
                    for blk in range(8):
                        t = scp * 8 + blk
                        for dt_i in range(DT):
                            nc.tensor.matmul(
                                out=ps[:, t:t + 1],
                                lhsT=hT[dt_i][:, blk * 128:(blk + 1) * 128],
                                rhs=v_col[:, dt_i:dt_i + 1],
                                start=(dt_i == 0), stop=(dt_i == DT - 1),
                            )

                # softmax pieces
                u_col = small_pool.tile([128, 16], f32r, tag="ucol")
                nc.scalar.activation(out=u_col, in_=ps, func=Exp)
                den_p = small_pool.tile([128, 1], f32r, tag="denp")
                with nc.allow_low_precision(reason="f32r is full-width fp32"):
                    nc.vector.tensor_reduce(out=den_p, in_=u_col, axis=AX, op=add)
                # fp32r matmuls need an even moving-dim count: broadcast the
                # column to width 2 via a stride-0 AP (2nd output col unused).
                den_b = bass.AP(tensor=den_p.tensor, offset=den_p.offset,
                                ap=[den_p.ap[0], [0, 2]])
                pden = pp_sm.tile([128, 16], f32, tag="sm")
                nc.tensor.matmul(out=pden[:, 0:2], lhsT=ones_r, rhs=den_b,
                                 start=True, stop=True)
                rden = small_pool.tile([128, 1], f32, tag="rden")
                nc.vector.reciprocal(out=rden, in_=pden[:, 0:1])

                # c_t columns: accumulate over the 16 s-tiles
                pc = pp_sm.tile([128, 16], f32, tag="sm")
                for ec in range(ET):
                    for t in range(16):
                        sc, scj = t // 4, t % 4
                        u1 = u_col[:, t:t + 1]
                        u2 = bass.AP(tensor=u1.tensor, offset=u1.offset,
                                     ap=[u1.ap[0], [0, 2]])
                        nc.tensor.matmul(
                            out=pc[:, 2 * ec:2 * ec + 2],
                            lhsT=en[sc][:, scj, ec * 128:(ec + 1) * 128],
                            rhs=u2,
                            start=(t == 0), stop=(t == 15),
                        )
                pc_str = bass.AP(tensor=pc.tensor, offset=pc.offset,
                                 ap=[pc.ap[0], [2, ET]])
                nc.vector.tensor_scalar_mul(
                    out=ct_all[:, b, :], in0=pc_str, scalar1=rden
                )

            # ---- output: transpose [128 p, (b ec)] -> [(b ec), p], one DMA ----
            pf = pp_t.tile([BPC * ET, 128], f32r, tag="pt")
            nc.tensor.transpose(
                out=pf,
                in_=ct_all.rearrange("p b e -> p (b e)"),
                identity=ident_r,
            )
            ct_out = const.tile([BPC * ET, 128], f32)
            nc.vector.tensor_copy(out=ct_out, in_=pf)
            nc.sync.dma_start(
                out=out.rearrange("b o (et p) -> (b o et) p", p=128),
                in_=ct_out,
            )

    nc.compile()
    return nc


def _get_nc():
    if "nc" not in _CACHE:
        _CACHE["nc"] = _build()
    return _CACHE["nc"]


def _run(inputs, trace=False, **kw):
    from concourse.bass_utils import run_bass_kernel_spmd

    nc = _get_nc()
    enc = np.asarray(inputs["enc_outs"], dtype=np.float32)
    ht = np.asarray(inputs["ht"], dtype=np.float32)
    W_w = np.asarray(inputs["W_w"], dtype=np.float32)
    W_b = np.asarray(inputs["W_b"], dtype=np.float32)
    V_w = np.asarray(inputs["V_w"], dtype=np.float32)
    in_maps = []
    for c in range(N_CORES):
        sl = slice(c * BPC, (c + 1) * BPC)
        in_maps.append({
            "enc_outs": enc[sl],
            "ht": ht[:, sl],
            "W_w": W_w,
            "W_b": W_b,
            "V_w": V_w,
        })
    res = run_bass_kernel_spmd(
        nc, in_maps, core_ids=list(range(N_CORES)), trace=trace, **kw
    )
    full = np.concatenate([res.results[c]["c_t"] for c in range(N_CORES)], axis=0)
    return full, res


def kernel(**inputs) -> np.ndarray:
    out, _ = _run(inputs, trace=False)
    return out
